# revision 14
# baseline (speedup 1.0000x reference)
"""CrossBandBiMamba Trainium2 kernel v3.

Columns per chunk are (k, t) k-major (host pre-transposes). 20 uniform chunks
of t=15 (cn=450). Scans run unrolled over chunk PAIRS at DVE 2x mode on
[CH, c=2, S, K, T] tiles; u is built directly in H (prefill + in-place mult).
"""
import numpy as np
from contextlib import ExitStack

CH, T, K, S, NCORES = 128, 300, 30, 16, 8
TKF = T * K  # 9000
TC = 15          # sequences per chunk
CN = TC * K      # 450 cols per chunk
NCHUNK = T // TC  # 20
G = 2            # chunks per scan group
SLOT = S * K * TC  # 7200 elems per chunk-slot

_CACHE = {}


def _host_prep(inputs):
    import ml_dtypes
    bf16 = ml_dtypes.bfloat16
    f32 = np.float32
    g = inputs['ln_g'].astype(f32)
    b = inputs['ln_b'].astype(f32)
    comb_W = inputs['comb_W'].astype(f32)
    W = {}
    for d in ('f', 'b'):
        Win = inputs[d + '_Win'].astype(f32)
        bin_ = inputs[d + '_bin'].astype(f32)
        convw = inputs[d + '_convw'].astype(f32)
        convb = inputs[d + '_convb'].astype(f32)
        Wp, Wzp = Win[:, :CH], Win[:, CH:]
        wconv = np.stack([(g[:, None] * Wp) * convw[:, tau][None, :]
                          for tau in range(4)], axis=1)
        W[d + '_wconv'] = wconv.reshape(CH, 4 * CH).astype(bf16)
        W[d + '_wz'] = (g[:, None] * Wzp).astype(bf16)
        bias1 = b @ Wp + bin_[:CH]
        W[d + '_bz'] = (b @ Wzp + bin_[CH:]).reshape(CH, 1).astype(f32)
        bc = np.stack([bias1 * convw[:, 3 - j:4].sum(1) + convb
                       for j in range(4)], axis=1)
        W[d + '_bconv'] = bc.astype(f32)
        wx = inputs[d + '_Wx'].astype(f32).copy()
        wx[:, 8:24] *= -1.0  # u = (ln(E)*xp)*(-B) since ln(E) = -dt
        W[d + '_wx'] = wx.astype(bf16)
        W[d + '_wdt'] = inputs[d + '_Wdt'].astype(f32).astype(bf16)
        W[d + '_nbdt'] = (-inputs[d + '_bdt'].astype(f32)).reshape(CH, 1)
        W[d + '_dp'] = inputs[d + '_D'].astype(f32).reshape(CH, 1)
        half = comb_W[:CH] if d == 'f' else comb_W[CH:]
        W[d + '_woc'] = (inputs[d + '_Wout'].astype(f32) @ half).astype(bf16)
    bias_comb = (inputs['comb_b'].astype(f32)
                 + inputs['f_bout'].astype(f32) @ comb_W[:CH]
                 + inputs['b_bout'].astype(f32) @ comb_W[CH:])
    W['bias_comb'] = bias_comb.reshape(CH, 1).astype(f32)
    return W


def _prep_x(xi):
    """(CH, T, K) f32 -> (CH, TKF) with per-chunk (k, t) order."""
    blocks = []
    for c in range(NCHUNK):
        blk = xi[:, c * TC:(c + 1) * TC, :]          # (CH, 15, 30)
        blocks.append(np.transpose(blk, (0, 2, 1)).reshape(CH, CN))
    return np.ascontiguousarray(np.concatenate(blocks, axis=1))


def _unprep_out(oi):
    """(CH, TKF) device layout -> (CH, T, K)."""
    out = np.empty((CH, T, K), np.float32)
    for c in range(NCHUNK):
        blk = oi[:, c * CN:(c + 1) * CN].reshape(CH, K, TC)
        out[:, c * TC:(c + 1) * TC, :] = np.transpose(blk, (0, 2, 1))
    return out


def _build():
    import concourse.bass as bass
    import concourse.tile as tile
    from concourse import mybir, bacc

    f32 = mybir.dt.float32
    bf = mybir.dt.bfloat16
    AF = mybir.ActivationFunctionType
    OP = mybir.AluOpType

    nc = bacc.Bacc("TRN2", target_bir_lowering=False, debug=False,
                   num_devices=NCORES)

    x_d = nc.dram_tensor("x", [CH, TKF], f32, kind="ExternalInput")
    o_d = nc.dram_tensor("out", [CH, TKF], f32, kind="ExternalOutput")
    wd = {}
    for d in ('f', 'b'):
        wd[d + '_wconv'] = nc.dram_tensor(d + "_wconv", [CH, 4 * CH], bf, kind="ExternalInput")
        wd[d + '_wz'] = nc.dram_tensor(d + "_wz", [CH, CH], bf, kind="ExternalInput")
        wd[d + '_wx'] = nc.dram_tensor(d + "_wx", [CH, 40], bf, kind="ExternalInput")
        wd[d + '_wdt'] = nc.dram_tensor(d + "_wdt", [8, CH], bf, kind="ExternalInput")
        wd[d + '_woc'] = nc.dram_tensor(d + "_woc", [CH, CH], bf, kind="ExternalInput")
        wd[d + '_bconv'] = nc.dram_tensor(d + "_bconv", [CH, 4], f32, kind="ExternalInput")
        for n in ('bz', 'nbdt', 'dp'):
            wd[f'{d}_{n}'] = nc.dram_tensor(f"{d}_{n}", [CH, 1], f32, kind="ExternalInput")
    wd['bias_comb'] = nc.dram_tensor("bias_comb", [CH, 1], f32, kind="ExternalInput")

    stats_dram = nc.dram_tensor("sp_stats", [2, TKF], f32)
    lnrow_dram = nc.dram_tensor("sp_lnrow", [2, TKF], bf)
    bc_dram = {d: nc.dram_tensor(f"sp_bc_{d}", [32, TKF], bf) for d in ('f', 'b')}

    with tile.TileContext(nc) as tc_, ExitStack() as ctx:
        tcx = tc_
        wpool = ctx.enter_context(tcx.tile_pool(name="w", bufs=1))
        big = ctx.enter_context(tcx.tile_pool(name="big", bufs=1))
        sbuf = ctx.enter_context(tcx.tile_pool(name="sb", bufs=2))
        scan_p = ctx.enter_context(tcx.tile_pool(name="scan", bufs=1))
        cpool = ctx.enter_context(tcx.tile_pool(name="cp", bufs=1))
        sb4 = ctx.enter_context(tcx.tile_pool(name="sb4", bufs=4))
        psA = ctx.enter_context(tcx.tile_pool(name="psA", bufs=1, space="PSUM"))
        psB = ctx.enter_context(tcx.tile_pool(name="psB", bufs=1, space="PSUM"))
        psO = ctx.enter_context(tcx.tile_pool(name="psO", bufs=1, space="PSUM"))

        wt = {}
        for name, dten in wd.items():
            shp = list(dten.shape)
            t = wpool.tile(shp, dten.dtype, name="w_" + name)
            nc.sync.dma_start(t[:], dten[:, :])
            wt[name] = t
        ones_c = wpool.tile([CH, 1], f32, name="ones_c")
        nc.vector.memset(ones_c[:], 1.0 / CH)

        # ================= P0: LN stats =================
        for c in range(NCHUNK):
            c0 = c * CN
            xch_t = sbuf.tile([CH, CN], f32, tag="xf32p0")
            nc.sync.dma_start(xch_t[:, :], x_d[:, c0:c0 + CN])
            sq = sbuf.tile([CH, CN], f32, tag="o_sb")
            nc.scalar.square(sq[:, :], xch_t[:, :])
            st_mu = psA.tile([1, CN], f32, tag="psX_f")
            nc.tensor.matmul(st_mu[0:1, :], ones_c[:], xch_t[:, :], start=True, stop=True)
            st_m2 = psA.tile([1, CN], f32, tag="psY_f")
            nc.tensor.matmul(st_m2[0:1, :], ones_c[:], sq[:, :], start=True, stop=True)
            st_sb = sbuf.tile([1, 2 * CN], f32, tag="p0st")
            nc.scalar.copy(st_sb[0:1, 0:CN], st_mu[0:1, :])
            nc.scalar.copy(st_sb[0:1, CN:2 * CN], st_m2[0:1, :])
            nc.sync.dma_start(stats_dram[0:1, c0:c0 + CN], st_sb[0:1, 0:CN])
            nc.sync.dma_start(stats_dram[1:2, c0:c0 + CN], st_sb[0:1, CN:2 * CN])
        s2 = big.tile([90, 200], f32, name="s2")
        nc.sync.dma_start(s2[:, 0:100], stats_dram[0:1, :])
        nc.sync.dma_start(s2[:, 100:200], stats_dram[1:2, :])
        mu2 = sbuf.tile([90, 100], f32, tag="mu2")
        nc.scalar.square(mu2[:], s2[:, 0:100])
        var = mu2
        nc.vector.tensor_sub(var[:], s2[:, 100:200], mu2[:])
        eps_t = wpool.tile([90, 1], f32, name="eps_t")
        nc.vector.memset(eps_t[:], 1e-5)
        std = sbuf.tile([90, 100], f32, tag="rstd")
        nc.scalar.activation(std[:], var[:], AF.Sqrt, bias=eps_t[:, 0:1], scale=1.0)
        rstd = sbuf.tile([90, 100], f32, tag="rstd")
        nc.vector.reciprocal(rstd[:], std[:])
        nmr = sbuf.tile([90, 100], f32, tag="nmr")
        nc.vector.scalar_tensor_tensor(nmr[:], s2[:, 0:100], -1.0, rstd[:],
                                       OP.mult, OP.mult)
        rstd_bf = sbuf.tile([90, 100], bf, tag="rstd_bf")
        nc.scalar.copy(rstd_bf[:], rstd[:])
        nmr_bf = sbuf.tile([90, 100], bf, tag="nmr_bf")
        nc.scalar.copy(nmr_bf[:], nmr[:])
        nc.sync.dma_start(lnrow_dram[0:1, :], rstd_bf[:])
        nc.sync.dma_start(lnrow_dram[1:2, :], nmr_bf[:])

        # ================= main fused loop over chunk PAIRS =================
        dA_all = scan_p.tile([CH, 2 * G * SLOT], bf, name="dA_all")
        H_all = scan_p.tile([CH, 2 * G * SLOT], bf, name="H_all")
        dA = {'f': dA_all[:, 0:G * SLOT], 'b': dA_all[:, G * SLOT:2 * G * SLOT]}
        H = {'f': H_all[:, 0:G * SLOT], 'b': H_all[:, G * SLOT:2 * G * SLOT]}

        for pair in range(NCHUNK // G):
            keep = {}
            for ci in range(G):
                c = G * pair + ci
                c0 = c * CN
                so = ci * SLOT
                # -- load x + LN rows --
                rstdb = sbuf.tile([CH, CN], bf, tag="rstdb")
                nc.sync.dma_start(rstdb[:, :],
                                  lnrow_dram[0:1, c0:c0 + CN].to_broadcast([CH, CN]))
                nmrb = sbuf.tile([CH, CN], bf, tag="nmrb")
                nc.sync.dma_start(nmrb[:, :],
                                  lnrow_dram[1:2, c0:c0 + CN].to_broadcast([CH, CN]))
                x_bf = sb4.tile([CH, CN], bf, tag="x_bf", name="x_bf")
                nc.gpsimd.dma_start(x_bf[:, :], x_d[:, c0:c0 + CN])
                xr = sbuf.tile([CH, CN], bf, tag="xr")
                nc.vector.tensor_mul(xr[:, :], x_bf[:, :], rstdb[:, :])
                nc.vector.tensor_add(nmrb[:, :], xr[:, :], nmrb[:, :])
                xn3 = nmrb[:, :].rearrange("p (k t) -> p k t", k=K)

                ck = {'x_bf': x_bf}
                # -- P1+P2+P3 per dir: conv/z matmuls, silu, xdbl, dt --
                for d in ('f', 'b'):
                    fwd = (d == 'f')
                    xnv = xn3 if fwd else xn3[:, ::-1, :]
                    xp_ps = psA.tile([CH, CN], f32, tag="psX_" + d, name="psX_" + d)
                    xp3 = xp_ps[:, :].rearrange("p (k t) -> p k t", k=K)
                    wc = wt[d + '_wconv'][:].rearrange("p (t c) -> p t c", c=CH)
                    for i, tau in enumerate((3, 2, 1, 0)):
                        sh = 3 - tau
                        rhs = xnv[:, 0:K - sh, :]
                        out = xp3[:, sh:K, :] if sh else xp3[:, :, :]
                        nc.tensor.matmul(out, wc[:, tau:tau + 1, :], rhs,
                                         start=(i == 0), stop=(i == 3))
                    z_ps = psA.tile([CH, CN], f32, tag="psY_" + d, name="psY_" + d)
                    nc.tensor.matmul(z_ps[:, :].rearrange("p (k t) -> p k t", k=K),
                                     wt[d + '_wz'][:], xnv, start=True, stop=True)
                    xp_sb = sb4.tile([CH, CN], bf, tag="xp_" + d, name="xp_" + d)
                    for (ka, kb, j) in ((0, 1, 0), (1, 2, 1), (2, 3, 2), (3, K, 3)):
                        nc.scalar.activation(xp_sb[:, ka * TC:kb * TC],
                                             xp_ps[:, ka * TC:kb * TC],
                                             AF.Silu, bias=wt[d + '_bconv'][:, j:j + 1],
                                             scale=1.0)
                    zs_sb = sb4.tile([CH, CN], bf, tag="zs_" + d, name="zs_" + d)
                    nc.scalar.activation(zs_sb[:, :], z_ps[:, :], AF.Silu,
                                         bias=wt[d + '_bz'][:, 0:1], scale=1.0)
                    xd_ps = psB.tile([40, CN], f32, tag="psD", name="psD")
                    nc.tensor.matmul(xd_ps[:, :], wt[d + '_wx'][:], xp_sb[:, :],
                                     start=True, stop=True)
                    xd_sb = sbuf.tile([40, CN], bf, tag="xd", name="xd_sb")
                    nc.vector.tensor_copy(xd_sb[:, :], xd_ps[:, :])
                    nc.sync.dma_start(bc_dram[d][:, c0:c0 + CN], xd_sb[8:40, :])
                    Cch = cpool.tile([CH, SLOT], bf, tag="C_" + d, name="C_" + d)
                    c3 = Cch[:].rearrange("p (s n) -> p s n", s=S)
                    nc.sync.dma_start(
                        c3, bc_dram[d][16:32, c0:c0 + CN].unsqueeze(0)
                        .to_broadcast([CH, S, CN]))
                    ck['C_' + d] = Cch
                    dtr_ps = psB.tile([CH, CN], f32, tag="psE_" + d, name="psE_" + d)
                    nc.tensor.matmul(dtr_ps[:, :], wt[d + '_wdt'][:],
                                     xd_sb[0:8, :], start=True, stop=True)
                    dtr_sb = sbuf.tile([CH, CN], bf, tag="dt_" + d, name="dtr_" + d)
                    nc.scalar.copy(dtr_sb[:, :], dtr_ps[:, :])
                    ck['dtr_' + d] = dtr_sb
                    ck['xp_' + d] = xp_sb
                    ck['zs_' + d] = zs_sb
                keep[ci] = ck
            # -- P4: sigmoid for all chunks+dirs (single table load) --
            for ci in range(G):
                so = ci * SLOT
                for d in ('f', 'b'):
                    nc.scalar.activation(dA[d][:, so:so + CN],
                                         keep[ci]['dtr_' + d][:, :],
                                         AF.Sigmoid, bias=wt[d + '_nbdt'][:, 0:1],
                                         scale=-1.0)
            # -- P5: ln + exp powers for all chunks+dirs (single table load) --
            for ci in range(G):
                so = ci * SLOT
                for d in ('f', 'b'):
                    dts = sbuf.tile([CH, CN], bf, tag="dt_" + d, name="dt_" + d)
                    nc.scalar.activation(dts[:, :], dA[d][:, so:so + CN], AF.Ln)
                    keep[ci]['dt_' + d] = dts
            for ci in range(G):
                so = ci * SLOT
                for d in ('f', 'b'):
                    for s in range(1, 8):
                        nc.scalar.activation(dA[d][:, so + s * CN:so + (s + 1) * CN],
                                             keep[ci]['dt_' + d][:, :], AF.Exp,
                                             scale=float(s + 1))
            # -- cascade + P6: u = v*B in H slots; C broadcast --
            for ci in range(G):
                c = G * pair + ci
                c0 = c * CN
                so = ci * SLOT
                for d in ('f', 'b'):
                    e8bc = dA[d][:, so + 7 * CN:so + 8 * CN].unsqueeze(1) \
                        .to_broadcast([CH, 8, CN])
                    lo = dA[d][:, so:so + 8 * CN].rearrange("p (s n) -> p s n", s=8)
                    hi = dA[d][:, so + 8 * CN:so + 16 * CN] \
                        .rearrange("p (s n) -> p s n", s=8)
                    nc.vector.tensor_tensor(hi, lo, e8bc, OP.mult)
                    v_sb = sbuf.tile([CH, CN], bf, tag="v", name="v_sb")
                    nc.vector.tensor_mul(v_sb[:, :], keep[ci]['dt_' + d][:, :],
                                         keep[ci]['xp_' + d][:, :])
                    h3 = H[d][:, so:so + SLOT].rearrange("p (s n) -> p s n", s=S)
                    nc.sync.dma_start(
                        h3, bc_dram[d][0:16, c0:c0 + CN].unsqueeze(0)
                        .to_broadcast([CH, S, CN]))
                    vbc = v_sb[:, :].unsqueeze(1).to_broadcast([CH, S, CN])
                    nc.vector.tensor_tensor(h3, vbc, h3, OP.mult)
            # -- P7/P8: per direction, scan then immediately read out (frees
            #    H[d]/dA[d] a phase earlier for the next pair's prefetch) --
            yg_all = {}
            for d in ('f', 'b'):
                H5 = H[d].rearrange("p (c s k t) -> p c s k t", c=G, s=S, k=K)
                A5 = dA[d].rearrange("p (c s k t) -> p c s k t", c=G, s=S, k=K)
                scr = cpool.tile([CH, G * S * TC], bf, tag="scr", name="scr")
                s4 = scr[:].rearrange("p (c s t) -> p c s t", c=G, s=S)
                for k in range(1, K):
                    nc.vector.tensor_tensor(s4, A5[:, :, :, k, :],
                                            H5[:, :, :, k - 1, :], OP.mult)
                    nc.vector.tensor_tensor(H5[:, :, :, k, :], s4,
                                            H5[:, :, :, k, :], OP.add)
                for ci in range(G):
                    so = ci * SLOT
                    ck = keep[ci]
                    Hs = H[d][:, so:so + SLOT]
                    nc.vector.tensor_tensor(Hs, Hs, ck['C_' + d][:, :], OP.mult)
                    nc.vector.tensor_tensor(H[d][:, so:so + 8 * CN],
                                            H[d][:, so:so + 8 * CN],
                                            H[d][:, so + 8 * CN:so + 16 * CN], OP.add)
                    nc.vector.tensor_tensor(H[d][:, so:so + 4 * CN],
                                            H[d][:, so:so + 4 * CN],
                                            H[d][:, so + 4 * CN:so + 8 * CN], OP.add)
                    nc.vector.tensor_tensor(H[d][:, so:so + 2 * CN],
                                            H[d][:, so:so + 2 * CN],
                                            H[d][:, so + 2 * CN:so + 4 * CN], OP.add)
                    y_sb = sbuf.tile([CH, CN], bf, tag="y_" + d, name="y_" + d)
                    nc.vector.tensor_tensor(y_sb[:, :], H[d][:, so:so + CN],
                                            H[d][:, so + CN:so + 2 * CN], OP.add)
                    t1 = sbuf.tile([CH, CN], bf, tag="t1_" + d, name="t1_" + d)
                    nc.vector.scalar_tensor_tensor(t1[:, :], ck['xp_' + d][:, :],
                                                   wt[d + '_dp'][:, 0:1], y_sb[:, :],
                                                   OP.mult, OP.add)
                    ygt = sbuf.tile([CH, CN], bf, tag="yg_" + d, name="yg_" + d)
                    nc.vector.tensor_mul(ygt[:, :], t1[:, :], ck['zs_' + d][:, :])
                    yg_all[(d, ci)] = ygt
            # -- output per chunk --
            for ci in range(G):
                c = G * pair + ci
                c0 = c * CN
                ck = keep[ci]
                o_ps = psO.tile([CH, CN], f32, tag="o_ps", name="o_ps")
                nc.tensor.matmul(o_ps[:, :], wt['f_woc'][:],
                                 yg_all[('f', ci)][:, :], start=True, stop=False)
                ygb3 = yg_all[('b', ci)][:, :].rearrange("p (k t) -> p k t", k=K)
                nc.tensor.matmul(o_ps[:, :].rearrange("p (k t) -> p k t", k=K),
                                 wt['b_woc'][:], ygb3[:, ::-1, :],
                                 start=False, stop=True)
                o_sb = sbuf.tile([CH, CN], f32, tag="o_sb", name="o_sb")
                nc.vector.scalar_tensor_tensor(o_sb[:, :], o_ps[:, :],
                                               wt['bias_comb'][:, 0:1],
                                               ck['x_bf'][:, :], OP.add, OP.add)
                nc.sync.dma_start(o_d[:, c0:c0 + CN], o_sb[:, :])

    nc.compile()
    return nc


def kernel(**inputs):
    from concourse.bass_utils import run_bass_kernel_spmd

    if 'nc' not in _CACHE:
        _CACHE['nc'] = _build()
    nc = _CACHE['nc']
    W = _host_prep(inputs)
    x = np.asarray(inputs['x'], dtype=np.float32)  # (8, 128, 300, 30)
    in_maps = []
    for i in range(NCORES):
        m = dict(W)
        m['x'] = _prep_x(x[i])
        in_maps.append(m)
    res = run_bass_kernel_spmd(nc, in_maps, core_ids=list(range(NCORES)))
    out = np.stack([_unprep_out(np.asarray(res.results[i]['out'], dtype=np.float32))
                    for i in range(NCORES)])
    return out
